# revision 1
# baseline (speedup 1.0000x reference)
"""Trainium2 Bass kernel for GCMC-style GNN message passing (nn_Net_6425271075083).

Strategy (8 NeuronCores, users sharded 1250/core):
  - Host densifies the edge lists into per-rating adjacency count matrices
    (exact in bf16), folds the degree norms into the feature/adjacency
    operands (cu into ufeat and into G^T's user columns, ci into ifeat),
    and pre-packs everything into a handful of wide DRAM tensors so the
    device does few, large HWDGE DMAs.
  - Device phases (all matmuls bf16, fp32 PSUM):
      A: XW products against W_cat = [W_1|...|W_5]:
         xh_k = (cu.ufeat)_k @ W_cat   [128, 1280]   (10 user k-tiles)
         hh_j = (ci.ifeat)_j @ W_cat   [128, 1280]   (8 item k-tiles)
      B: item partials  M^T += xh_k[r-slice]^T @ A_r[k]   -> mcT [256,1024]
         -> DRAM -> AllReduce over 8 cores (loop/timing mode: same-size
         HWDGE DMA copy) -> itemagg
      C: user aggregates, computed transposed so no PE transposes needed:
         psU[a-tile] += hh_j[r-slice]^T @ (D_cu G_r^T)[j]  -> [256, 1280]
         leaky -> actT
      D: heads: sT = fcw^T @ actT + Y0^T @ histT (+fcb); extra row = bu+gm
         qT = fcw^T @ leaky(itemagg) (+fcb); extra row = 1/ci
      E: out = D_ci (qT.T @ sT) + bi  evicted bf16 (host casts to f32)
"""
import numpy as np
import ml_dtypes

import concourse.bass as bass
import concourse.bacc as bacc
import concourse.mybir as mybir
import concourse.tile as tile
from concourse import bass_utils

BF = ml_dtypes.bfloat16
F32 = mybir.dt.float32
BF16 = mybir.dt.bfloat16

N_CORES = 8
U, I, R, D, O, H = 10000, 1000, 5, 256, 64, 1001
UC = U // N_CORES          # 1250
UCP = 1280                 # users per core, padded
IP = 1024                  # items padded
HP = 1024                  # hist bins padded
KU = UCP // 128            # 10 user k-tiles
KI = IP // 128             # 8 item k-tiles
KH = HP // 128             # 8 hist k-tiles
KD = D // 128              # 2 feature k-tiles
RA = R * D                 # 1280: all ratings' agg units concatenated
UCHUNKS = [(0, 512), (512, 512), (1024, 256)]   # over UCP
ICHUNKS = [(0, 512), (512, 512)]                # over IP
RCHUNKS = [(0, 512), (512, 512), (1024, 256)]   # over RA

_ALU = mybir.AluOpType
_ACT = mybir.ActivationFunctionType


def host_preprocess(src_idx, dst_idx, implicit_matrix, sqrt_count, global_mean,
                    ufeat, ifeat, W, fc_w, fc_b, bu, bi, Y):
    """Layout/sharding only plus degree/adjacency densification; the NN math
    (feature transforms, aggregation, heads) happens on device."""
    src = np.asarray(src_idx).astype(np.int64)
    dst = np.asarray(dst_idx).astype(np.int64)
    im = np.asarray(implicit_matrix).astype(np.int64)
    sqrt_count = np.asarray(sqrt_count, np.float32)
    gm = float(np.asarray(global_mean, np.float32).reshape(1)[0])
    ufeat = np.asarray(ufeat, np.float32)
    ifeat = np.asarray(ifeat, np.float32)
    W = np.asarray(W, np.float32)
    fc_w = np.asarray(fc_w, np.float32)
    fc_b = np.asarray(fc_b, np.float32)
    bu = np.asarray(bu, np.float32)
    bi = np.asarray(bi, np.float32)
    Y = np.asarray(Y, np.float32)

    deg_u = np.bincount(src.reshape(-1), minlength=U).astype(np.float32)
    deg_i = np.bincount(dst.reshape(-1), minlength=I).astype(np.float32)
    cu = 1.0 / np.sqrt(np.maximum(deg_u, 1.0))
    ci = 1.0 / np.sqrt(np.maximum(deg_i, 1.0))

    def pack_cols(vec, ntiles):
        padded = np.zeros(128 * ntiles, np.float32)
        padded[:len(vec)] = vec
        return np.ascontiguousarray(padded.reshape(ntiles, 128).T)

    # [128, 17]: cols 0:8 ci2, 8:16 bi2, col 16 rows 0:64 = fc_b
    cpack = np.zeros((128, 17), np.float32)
    cpack[:, 0:KI] = pack_cols(ci, KI)
    cpack[:, KI:2 * KI] = pack_cols(bi[:, 0], KI)
    cpack[0:O, 16] = fc_b

    # item-side consts, replicated per core
    iftcat = np.zeros((128, KD * IP), BF)
    ifn = (ifeat * ci[:, None]).T              # [256, 1000]
    for kk in range(KD):
        iftcat[:, kk * IP:kk * IP + I] = ifn[kk * 128:(kk + 1) * 128].astype(BF)

    wcat = np.zeros((128, KD * RA), BF)
    for kk in range(KD):
        for r in range(R):
            wcat[:, kk * RA + r * D:kk * RA + (r + 1) * D] = \
                W[r, kk * 128:(kk + 1) * 128, :].astype(BF)

    fcwb = np.zeros((128, KD * O), BF)
    for kk in range(KD):
        fcwb[:, kk * O:(kk + 1) * O] = fc_w[kk * 128:(kk + 1) * 128, :].astype(BF)

    Y0 = Y.copy()
    Y0[0] = 0.0
    y0cat = np.zeros((128, KH * O), BF)
    for kh in range(KH):
        rows = min(128, H - kh * 128)
        y0cat[:rows, kh * O:(kh + 1) * O] = Y0[kh * 128:kh * 128 + rows].astype(BF)

    # dense adjacency counts per rating [U, I]
    G = np.zeros((R, U, I), np.float32)
    for r in range(R):
        G[r] = np.bincount(src[r] * I + dst[r], minlength=U * I).reshape(U, I)

    # implicit histogram [U, H] with 1/sqrt_count folded
    hist = np.bincount((np.arange(U)[:, None] * H + im).reshape(-1),
                       minlength=U * H).reshape(U, H).astype(np.float32)
    histp = hist / sqrt_count

    in_maps = []
    for c in range(N_CORES):
        us = slice(c * UC, (c + 1) * UC)
        cu_c = cu[us]

        # ga2[k, p, r*IP + i] = G_r[k*128+p, i]  (users on partitions, raw counts)
        ga2 = np.zeros((KU, 128, R * IP), BF)
        Gc = G[:, us, :]                        # [R, UC, I]
        for k in range(KU):
            rows = min(128, UC - k * 128)
            for r in range(R):
                ga2[k, :rows, r * IP:r * IP + I] = \
                    Gc[r, k * 128:k * 128 + rows].astype(BF)

        # gbcat[j, p, r*UCP + u] = G_r[u, j*128+p] * cu[u]  (items on partitions)
        gbcat = np.zeros((KI, 128, R * UCP), BF)
        for r in range(R):
            gt = (Gc[r] * cu_c[:, None]).T.astype(BF)   # [I, UC]
            for j in range(KI):
                rows = min(128, I - j * 128)
                gbcat[j, :rows, r * UCP:r * UCP + UC] = gt[j * 128:j * 128 + rows]

        uftcat = np.zeros((128, KD * UCP), BF)
        ufn = (ufeat[us] * cu_c[:, None]).T     # [256, UC]
        for kk in range(KD):
            uftcat[:, kk * UCP:kk * UCP + UC] = ufn[kk * 128:(kk + 1) * 128].astype(BF)

        histcat = np.zeros((128, KH * UCP), BF)
        hpt = histp[us].T                       # [H, UC]
        for kh in range(KH):
            rows = min(128, H - kh * 128)
            histcat[:rows, kh * UCP:kh * UCP + UC] = \
                hpt[kh * 128:kh * 128 + rows].astype(BF)

        # crow row 0 = 1/ci (qT bias row), row 1 = bu + gm (sT bias row)
        crow = np.zeros((2, UCP), np.float32)
        crow[0, :I] = 1.0 / ci
        crow[1, :UC] = bu[us, 0] + gm

        in_maps.append({
            "ga2": ga2, "gbcat": gbcat,
            "uftcat": uftcat, "iftcat": iftcat,
            "wcat": wcat, "fcwb": fcwb, "y0cat": y0cat,
            "histcat": histcat, "cpack": cpack, "crow": crow,
        })
    return in_maps


def declare_io(nc, timing_mode=False):
    t = {}
    def inp(name, shape, dt):
        t[name] = nc.dram_tensor(name, list(shape), dt, kind="ExternalInput").ap()
    inp("ga2", (KU, 128, R * IP), BF16)
    inp("gbcat", (KI, 128, R * UCP), BF16)
    inp("uftcat", (128, KD * UCP), BF16)
    inp("iftcat", (128, KD * IP), BF16)
    inp("wcat", (128, KD * RA), BF16)
    inp("fcwb", (128, KD * O), BF16)
    inp("y0cat", (128, KH * O), BF16)
    inp("histcat", (128, KH * UCP), BF16)
    inp("cpack", (128, 17), F32)
    inp("crow", (2, UCP), F32)
    if timing_mode:
        t["tick"] = nc.dram_tensor("tick", [1, 4], BF16, kind="ExternalOutput").ap()
    else:
        t["out"] = nc.dram_tensor("out", [I, UC], BF16, kind="ExternalOutput").ap()
    return t


def emit_body(nc, tc, t, it, timing_mode=False, loop_mode=False,
              phases="ABHCDE"):
    """Emit one full compute pass. `it` suffixes tile names for repeats.

    `phases` selects which blocks to emit (HW phase-timing probes):
    A=user XW, B=item partials, H=item-side XW, C=user aggregates,
    D=heads, E=final matmul+out.
    """
    from contextlib import ExitStack
    ctx = ExitStack()
    P = 128
    phA, phB, phH = "A" in phases, "B" in phases, "H" in phases
    phC, phD = "C" in phases, "D" in phases
    phE = "E" in phases or "e" in phases   # final matmul+evict
    phEdma = "E" in phases                 # include the out DMA

    const = ctx.enter_context(tc.tile_pool(name=f"const{it}", bufs=1))

    def loadc(name, shape, dt, src_ap, eng):
        tl = const.tile(shape, dt, name=f"{name}{it}")
        eng.dma_start(tl[:], src_ap)
        return tl

    # consts on the ACT ring; the SP ring starts on ga2 immediately.
    # Emission order = issue order: phase-A operands first, hist last
    # (only needed in phase D; it loads on SP after the ga tiles).
    uft = loadc("uft", [P, KD * UCP], BF16, t["uftcat"][:], nc.sync)
    wc = loadc("wc", [P, KD * RA], BF16, t["wcat"][:], nc.scalar)
    ift = loadc("ift", [P, KD * IP], BF16, t["iftcat"][:], nc.scalar)
    fcw = loadc("fcw", [P, KD * O], BF16, t["fcwb"][:], nc.scalar)
    cp = loadc("cp", [P, 17], F32, t["cpack"][:], nc.scalar)
    cr0 = const.tile([1, UCP], F32, name=f"cr0{it}")
    cr1 = const.tile([1, UCP], F32, name=f"cr1{it}")
    y0 = loadc("y0", [P, KH * O], BF16, t["y0cat"][:], nc.scalar)
    hist = const.tile([P, KH * UCP], BF16, name=f"hist{it}")

    ci2 = cp[:, 0:KI]
    bi2 = cp[:, KI:2 * KI]
    fcb = cp[0:O, 16:17]

    # alternate PSUM->SBUF evictions between DVE and ACT
    _evict_flip = [0]
    def evict_copy(dst_ap, src_ap):
        if _evict_flip[0] % 2 == 0:
            nc.vector.tensor_copy(dst_ap, src_ap)
        else:
            nc.scalar.activation(dst_ap, src_ap, _ACT.Identity)
        _evict_flip[0] += 1

    # ---------------- phase A: XW products against W_cat ----------------
    xh = [const.tile([P, RA], BF16, name=f"xh{k}{it}") for k in range(KU)]
    hh = [const.tile([P, RA], BF16, name=f"hh{j}{it}") for j in range(KI)]
    pa = ExitStack()
    psxw = pa.enter_context(tc.tile_pool(name=f"psxw{it}", bufs=4, space="PSUM"))

    def emit_xw(dst, src_sb, src_w):
        # dst[u, r*D+a] = sum_kk src_sb[kk-tile]^T @ W_cat[kk-tile]
        for (c0, cw) in RCHUNKS:
            ps = psxw.tile([P, 512], F32, name=f"psxw{it}", tag="psxw")
            for kk in range(KD):
                nc.tensor.matmul(ps[:, 0:cw], src_sb[kk],
                                 wc[:, kk * RA + c0:kk * RA + c0 + cw],
                                 start=(kk == 0), stop=(kk == KD - 1))
            evict_copy(dst[:, c0:c0 + cw], ps[:, 0:cw])

    if phA:
        for k in range(KU):
            emit_xw(xh[k], [uft[:, kk * UCP + k * P:kk * UCP + (k + 1) * P]
                            for kk in range(KD)], wc)

    # ---------------- phase B: item partials ----------------
    dram = ctx.enter_context(tc.tile_pool(name=f"dram{it}", bufs=1, space="DRAM"))
    itemp = dram.tile([D, IP], F32, name=f"itemp{it}")
    itemagg = dram.tile([D, IP], F32, name=f"itemagg{it}",
                        addr_space="Local" if loop_mode else "Shared")
    mcT = [const.tile([P, IP], F32, name=f"mcT{h}{it}") for h in range(2)]
    if phB:
        pb = ExitStack()
        psb = pb.enter_context(tc.tile_pool(name=f"psb{it}", bufs=1, space="PSUM"))
        ga_pool = ctx.enter_context(tc.tile_pool(name=f"ga{it}", bufs=2))
        psB = [[psb.tile([P, 512], F32, name=f"psB{h}{cix}{it}")
                for cix in range(2)] for h in range(2)]
        for k in range(KU):
            ga_t = ga_pool.tile([P, R * IP], BF16, name=f"ga_t{it}")
            nc.sync.dma_start(ga_t[:], t["ga2"][k])
            for r in range(R):
                for h in range(2):
                    lhsT = xh[k][:, r * D + h * P:r * D + (h + 1) * P]
                    for cix, (c0, cw) in enumerate(ICHUNKS):
                        nc.tensor.matmul(psB[h][cix][:], lhsT,
                                         ga_t[:, r * IP + c0:r * IP + c0 + cw],
                                         start=(k == 0 and r == 0),
                                         stop=(k == KU - 1 and r == R - 1))
        for h in range(2):
            for cix, (c0, cw) in enumerate(ICHUNKS):
                evict_copy(mcT[h][:, c0:c0 + cw], psB[h][cix][:])

    # hh products emitted here: they fill the PE pipeline while phase B's
    # PSUM banks drain and phase C's pool allocates
    if phH:
        for j in range(KI):
            emit_xw(hh[j], [ift[:, kk * IP + j * P:kk * IP + (j + 1) * P]
                            for kk in range(KD)], wc)

    if phB:
        for h in range(2):
            nc.sync.dma_start(itemp[h * P:(h + 1) * P, :], mcT[h][:])
        if not loop_mode:
            nc.gpsimd.collective_compute(
                "AllReduce", _ALU.add,
                replica_groups=[list(range(N_CORES))],
                ins=[itemp.opt()], outs=[itemagg.opt()],
            )
        pb.close()
    pa.close()

    # ---------------- q head (hidden under phase C) ----------------
    # Opened before psu so its 2 banks coexist with psu's 6; the q head
    # depends only on the item phase, so its chain runs under C's matmuls.
    pdq = ExitStack()
    pss = pdq.enter_context(tc.tile_pool(name=f"pss{it}", bufs=2, space="PSUM"))
    qT = const.tile([O + 1, IP], BF16, name=f"qT{it}")
    sT = const.tile([O + 1, UCP], BF16, name=f"sT{it}")
    if phD:
        nc.gpsimd.dma_start(cr0[:], t["crow"][0:1, :])
        nc.gpsimd.dma_start(cr1[:], t["crow"][1:2, :])
        iag = [const.tile([P, IP], F32, name=f"iag{kk}{it}") for kk in range(2)]
        qact = [const.tile([P, IP], BF16, name=f"qact{kk}{it}") for kk in range(2)]
        iag_src = itemp if loop_mode else itemagg
        for kk in range(2):
            nc.sync.dma_start(iag[kk][:], iag_src[kk * P:(kk + 1) * P, :])
            if loop_mode:
                nc.sync.dma_start(itemagg[kk * P:(kk + 1) * P, :], iag[kk][:])
            nc.vector.scalar_tensor_tensor(qact[kk][:], iag[kk][:], 0.1,
                                           iag[kk][:], _ALU.mult, _ALU.max)
        for (c0, cw) in ICHUNKS:
            psQ = pss.tile([O, 512], F32, name=f"psQ{it}", tag="pss")
            for kk in range(2):
                nc.tensor.matmul(psQ[:, 0:cw], fcw[:, kk * O:(kk + 1) * O],
                                 qact[kk][:, c0:c0 + cw],
                                 start=(kk == 0), stop=(kk == 1))
            nc.scalar.activation(qT[0:O, c0:c0 + cw], psQ[:, 0:cw],
                                 _ACT.Identity, bias=fcb, scale=1.0)
        nc.vector.tensor_copy(qT[O:O + 1, :], cr0[:, 0:IP])
        nc.vector.tensor_copy(sT[O:O + 1, :], cr1[:, 0:UCP])
    else:
        nc.vector.memset(qT[:], 0.0)
        nc.vector.memset(sT[:], 0.0)

    # ---------------- phase C: user aggregates (transposed) ----------------
    actT = [const.tile([P, UCP], BF16, name=f"actT{a}{it}") for a in range(2)]
    psSe = {}
    if phC:
        pc = ExitStack()
        psu = pc.enter_context(tc.tile_pool(name=f"psu{it}", bufs=1, space="PSUM"))
        gb_pool = ctx.enter_context(tc.tile_pool(name=f"gb{it}", bufs=3))
        psU = [[psu.tile([P, 512], F32, name=f"psU{a}{ci_}{it}")
                for ci_ in range(3)] for a in range(2)]
        for j in range(KI):
            gb_t = gb_pool.tile([P, R * UCP], BF16, name=f"gb_t{it}")
            nc.gpsimd.dma_start(gb_t[:], t["gbcat"][j])
            if j == 1:
                nc.gpsimd.dma_start(hist[:], t["histcat"][:])
            for r in range(R):
                for a in range(2):
                    lhsT = hh[j][:, r * D + a * P:r * D + (a + 1) * P]
                    for ci_, (c0, cw) in enumerate(UCHUNKS):
                        nc.tensor.matmul(psU[a][ci_][:, 0:cw], lhsT,
                                         gb_t[:, r * UCP + c0:r * UCP + c0 + cw],
                                         start=(j == 0 and r == 0),
                                         stop=(j == KI - 1 and r == R - 1))
        # leaky(user_agg^T) -> actT bf16  (cu already folded into gbcat).
        # HW allows only one PSUM read per DVE op, so evict to SBUF (ACT)
        # then leaky (DVE); chunk-major so phase D starts after two chunks.
        uag = [const.tile([P, UCP], BF16, name=f"uag{a}{it}") for a in range(2)]
        for ci_, (c0, cw) in enumerate(UCHUNKS):
            for a in range(2):
                nc.scalar.activation(uag[a][:, c0:c0 + cw],
                                     psU[a][ci_][:, 0:cw], _ACT.Identity)
                nc.vector.scalar_tensor_tensor(actT[a][:, c0:c0 + cw],
                                               uag[a][:, c0:c0 + cw], 0.1,
                                               uag[a][:, c0:c0 + cw],
                                               _ALU.mult, _ALU.max)
        pc.close()
    elif phD:
        nc.scalar.dma_start(hist[:], t["histcat"][:])
        for a in range(2):
            nc.vector.memset(actT[a][:], 0.0)
    if not phB and phD:
        for h in range(2):
            nc.vector.memset(mcT[h][:], 0.0)
            nc.sync.dma_start(itemp[h * P:(h + 1) * P, :], mcT[h][:])

    if not (phD or phE):
        # timing probes without the head/final phases: tick from the last
        # active phase's bf16 output
        pdq.close()
        if timing_mode:
            src = (actT[1] if phC else hh[KI - 1] if phH
                   else xh[KU - 1] if phA else mcT[1])
            if src is mcT[1]:
                tickt = const.tile([1, 4], BF16, name=f"tickt{it}")
                nc.vector.tensor_copy(tickt[:], mcT[1][0:1, 0:4])
                src = tickt
            nc.sync.dma_start(t["tick"][:], src[0:1, 0:4])
        ctx.close()
        return

    # ---------------- phases D+E: heads + final, chunk-interleaved ----------
    pd = ExitStack()
    pso_pool = pd.enter_context(tc.tile_pool(name=f"pso{it}", bufs=6, space="PSUM"))
    out_pool = ctx.enter_context(tc.tile_pool(name=f"outp{it}", bufs=8))
    out_rows = [None] * KI

    if timing_mode:
        out_dst = dram.tile([I, UC], BF16, name=f"outscratch{it}")
    else:
        out_dst = t["out"]
    last_out_t = None
    for ci_, (c0, cw) in enumerate(UCHUNKS):
        if phD:
            psS = pss.tile([O, 512], F32, name=f"psS{it}", tag="pss")
            nmm = KD + KH
            i = 0
            # hist matmuls first: they depend only on hist, so they run
            # while this chunk's actT eviction is still draining
            for kh in range(KH):
                nc.tensor.matmul(psS[:, 0:cw], y0[:, kh * O:(kh + 1) * O],
                                 hist[:, kh * UCP + c0:kh * UCP + c0 + cw],
                                 start=(i == 0), stop=(i == nmm - 1))
                i += 1
            for kk in range(KD):
                nc.tensor.matmul(psS[:, 0:cw], fcw[:, kk * O:(kk + 1) * O],
                                 actT[kk][:, c0:c0 + cw],
                                 start=(i == 0), stop=(i == nmm - 1))
                i += 1
            nc.scalar.activation(sT[0:O, c0:c0 + cw], psS[:, 0:cw],
                                 _ACT.Identity, bias=fcb, scale=1.0)
        # final output for this user chunk while the next chunk's head runs;
        # per-mi rows accumulate across chunks and ship as one DMA each
        vw = min(cw, max(0, UC - c0))
        if vw <= 0 or not phE:
            continue
        last_chunk = (c0 + cw >= UC)
        for mi in range(KI):
            rows = min(P, I - mi * P)
            if rows <= 0:
                break
            psO = pso_pool.tile([P, 512], F32, name=f"psO{it}")
            nc.tensor.matmul(psO[:, 0:cw], qT[:, mi * P:(mi + 1) * P],
                             sT[:, c0:c0 + cw], start=True, stop=True)
            if c0 == 0:
                out_rows[mi] = out_pool.tile([P, UCP], BF16,
                                             name=f"out_t{mi}{it}", tag="out_t")
            out_t = out_rows[mi]
            if mi % 2 == 0:
                nc.scalar.activation(out_t[:, c0:c0 + cw], psO[:, 0:cw],
                                     _ACT.Identity, bias=bi2[:, mi:mi + 1],
                                     scale=ci2[:, mi:mi + 1])
            else:
                nc.vector.tensor_scalar(out_t[:, c0:c0 + cw], psO[:, 0:cw],
                                        ci2[:, mi:mi + 1], bi2[:, mi:mi + 1],
                                        _ALU.mult, _ALU.add)
            if phEdma and last_chunk:
                nc.scalar.dma_start(out_dst[mi * P:mi * P + rows, 0:UC],
                                    out_t[0:rows, 0:UC])
            last_out_t = out_t
    pd.close()
    pdq.close()
    if timing_mode:
        if last_out_t is not None:
            nc.sync.dma_start(t["tick"][:], last_out_t[0:1, 0:4])
        else:
            nc.sync.dma_start(t["tick"][:], sT[0:1, 0:4])
    ctx.close()


_PROGRAM_CACHE = {}


def build_program(repeat=1, timing_mode=False):
    key = (repeat, timing_mode)
    if key in _PROGRAM_CACHE:
        return _PROGRAM_CACHE[key]
    nc = bacc.Bacc("TRN2", target_bir_lowering=False, debug=False,
                   num_devices=N_CORES)
    t = declare_io(nc, timing_mode)
    with tile.TileContext(nc) as tc:
        for it in range(repeat):
            emit_body(nc, tc, t, f"_i{it}" if repeat > 1 else "",
                      timing_mode=timing_mode)
    nc.compile()
    _PROGRAM_CACHE[key] = (nc, t)
    return nc, t


def build_loop_program(trips, phases="ABHCDE"):
    key = ("loop", trips, phases)
    if key in _PROGRAM_CACHE:
        return _PROGRAM_CACHE[key]
    nc = bacc.Bacc("TRN2", target_bir_lowering=False, debug=False,
                   num_devices=N_CORES)
    t = declare_io(nc, timing_mode=True)
    with tile.TileContext(nc) as tc:
        with tc.For_i(0, trips, 1):
            emit_body(nc, tc, t, "", timing_mode=True, loop_mode=True,
                      phases=phases)
    nc.compile()
    _PROGRAM_CACHE[key] = (nc, t)
    return nc, t


def kernel(**inputs):
    in_maps = host_preprocess(**inputs)
    nc, _ = build_program()
    res = bass_utils.run_bass_kernel_spmd(
        nc, in_maps, core_ids=list(range(N_CORES)), trace=False)
    out = np.concatenate([res.results[c]["out"] for c in range(N_CORES)], axis=1)
    return out.astype(np.float32)



# revision 12
# speedup vs baseline: 1.4961x; 1.4961x over previous
"""Trainium2 Bass kernel for GCMC-style GNN message passing (nn_Net_6425271075083).

Strategy (8 NeuronCores, users sharded 1250/core), v2:
  - Host densifies edge lists into per-rating adjacency count matrices in
    fp8e4 (counts are small ints -> exact), packed in DoubleRow pair layout
    so the PE contracts 256 rows per pass at the fp8 double rate.
  - Device phases:
      A: xh_k = (cu.ufeat)_k @ W_cat  (bf16), evicted to fp8 (scale 16)
      B: item partials mcT += DoubleRow(xh8_t, ga8_t)  -> bf16 -> AllReduce
      H: hh_j = (ci.ifeat)_j @ W_cat  (bf16), evicted to fp8 (scale 256)
      C: user aggregates psU += DoubleRow(hh8_jt, gb8_jt); leaky -> actT
      D: heads: sT = cu*(fcw^T actT) + (Y0^T histT)/(32*2048) (+fcb);
         qT = fcw^T leaky(itemagg); bias rows from crow
      E: out = D_ci (qT.T @ sT) + bi, single wide DMA out (bf16)
  - DMA split across both HWDGE queues (sync + scalar); small consts on
    gpsimd SWDGE.
"""
import numpy as np
import ml_dtypes

import concourse.bass as bass
import concourse.bacc as bacc
import concourse.mybir as mybir
import concourse.tile as tile
from concourse import bass_utils

BF = ml_dtypes.bfloat16
F32 = mybir.dt.float32
BF16 = mybir.dt.bfloat16
FP8E4 = mybir.dt.float8e4
E4 = mybir.dt.np(FP8E4)

N_CORES = 8
U, I, R, D, O, H = 10000, 1000, 5, 256, 64, 1001
UC = U // N_CORES          # 1250
UCP = 1280                 # users per core, padded
IP = 1024                  # items padded
HP = 1024                  # hist bins padded
KU = UCP // 128            # 10 user k-tiles
KI = IP // 128             # 8 item k-tiles
KH = HP // 128             # 8 hist k-tiles
KD = D // 128              # 2 feature k-tiles
TU = KU // 2               # 5 user pair-tiles (DoubleRow)
TI = KI // 2               # 4 item pair-tiles
TH = KH // 2               # 4 hist pair-tiles
RA = R * D                 # 1280
SX = 16.0                  # xh fp8 scale
SHH = 256.0                # hh fp8 scale
SHI = 32.0                 # hist fp8 scale
SY = 2048.0                # y0 fp8 scale
UCHUNKS = [(0, 512), (512, 512), (1024, 256)]   # over UCP
ICHUNKS = [(0, 512), (512, 512)]                # over IP
RCHUNKS = [(0, 512), (512, 512), (1024, 256)]   # over RA

_ALU = mybir.AluOpType
_ACT = mybir.ActivationFunctionType
_DR = mybir.MatmulPerfMode.DoubleRow


def host_preprocess(src_idx, dst_idx, implicit_matrix, sqrt_count, global_mean,
                    ufeat, ifeat, W, fc_w, fc_b, bu, bi, Y):
    """Layout/sharding only plus degree/adjacency densification; the NN math
    (feature transforms, aggregation, heads) happens on device."""
    src = np.asarray(src_idx).astype(np.int64)
    dst = np.asarray(dst_idx).astype(np.int64)
    im = np.asarray(implicit_matrix).astype(np.int64)
    sqrt_count = np.asarray(sqrt_count, np.float32)
    gm = float(np.asarray(global_mean, np.float32).reshape(1)[0])
    ufeat = np.asarray(ufeat, np.float32)
    ifeat = np.asarray(ifeat, np.float32)
    W = np.asarray(W, np.float32)
    fc_w = np.asarray(fc_w, np.float32)
    fc_b = np.asarray(fc_b, np.float32)
    bu = np.asarray(bu, np.float32)
    bi = np.asarray(bi, np.float32)
    Y = np.asarray(Y, np.float32)

    deg_u = np.bincount(src.reshape(-1), minlength=U).astype(np.float32)
    deg_i = np.bincount(dst.reshape(-1), minlength=I).astype(np.float32)
    cu = 1.0 / np.sqrt(np.maximum(deg_u, 1.0))
    ci = 1.0 / np.sqrt(np.maximum(deg_i, 1.0))

    def pack_cols(vec, ntiles):
        padded = np.zeros(128 * ntiles, np.float32)
        padded[:len(vec)] = vec
        return np.ascontiguousarray(padded.reshape(ntiles, 128).T)

    # [128, 17]: cols 0:8 ci2, 8:16 bi2, col 16 rows 0:64 = fc_b
    cpack = np.zeros((128, 17), np.float32)
    cpack[:, 0:KI] = pack_cols(ci, KI)
    cpack[:, KI:2 * KI] = pack_cols(bi[:, 0], KI)
    cpack[0:O, 16] = fc_b

    # item-side consts, replicated per core
    iftcat = np.zeros((128, KD * IP), BF)
    ifn = (ifeat * ci[:, None]).T              # [256, 1000]
    for kk in range(KD):
        iftcat[:, kk * IP:kk * IP + I] = ifn[kk * 128:(kk + 1) * 128].astype(BF)

    wcat = np.zeros((128, KD * RA), BF)
    for kk in range(KD):
        for r in range(R):
            wcat[:, kk * RA + r * D:kk * RA + (r + 1) * D] = \
                W[r, kk * 128:(kk + 1) * 128, :].astype(BF)

    fcwb = np.zeros((128, KD * O), BF)
    for kk in range(KD):
        fcwb[:, kk * O:(kk + 1) * O] = fc_w[kk * 128:(kk + 1) * 128, :].astype(BF)

    Y0 = Y.copy()
    Y0[0] = 0.0
    # y08[p, q, j, o] = Y0[(2q+j)*128+p, o] * SY   (fp8 pair layout)
    ypad = np.zeros((TH * 2 * 128, O), np.float32)
    ypad[:H] = Y0 * SY
    y08 = np.ascontiguousarray(
        ypad.reshape(TH, 2, 128, O).transpose(2, 0, 1, 3).reshape(128, TH * 2 * O)
    ).astype(E4)

    # dense adjacency counts per rating [U, I]
    G = np.zeros((R, U, I), np.float32)
    for r in range(R):
        G[r] = np.bincount(src[r] * I + dst[r], minlength=U * I).reshape(U, I)

    # implicit histogram [U, H] with 1/sqrt_count folded
    hist = np.bincount((np.arange(U)[:, None] * H + im).reshape(-1),
                       minlength=U * H).reshape(U, H).astype(np.float32)
    histp = hist / sqrt_count

    in_maps = []
    for c in range(N_CORES):
        us = slice(c * UC, (c + 1) * UC)
        cu_c = cu[us]
        Gc = G[:, us, :]                        # [R, UC, I]

        # ga8[t][p, r, j, i] = G_r[(2t+j)*128+p, i]  (raw counts, fp8 exact)
        gpad = np.zeros((R, TU * 2 * 128, IP), np.float32)
        gpad[:, :UC, :I] = Gc
        ga8 = np.ascontiguousarray(
            gpad.reshape(R, TU, 2, 128, IP).transpose(1, 3, 0, 2, 4)
            .reshape(TU, 128, R * 2 * IP)).astype(E4)

        # gb8[jt][p, r, j, u] = G_r[u, (2jt+j)*128+p]  (raw counts)
        gtp = np.zeros((R, TI * 2 * 128, UCP), np.float32)
        gtp[:, :I, :UC] = Gc.transpose(0, 2, 1)
        gb8 = np.ascontiguousarray(
            gtp.reshape(R, TI, 2, 128, UCP).transpose(1, 3, 0, 2, 4)
            .reshape(TI, 128, R * 2 * UCP)).astype(E4)

        uftcat = np.zeros((128, KD * UCP), BF)
        ufn = (ufeat[us] * cu_c[:, None]).T     # [256, UC]
        for kk in range(KD):
            uftcat[:, kk * UCP:kk * UCP + UC] = ufn[kk * 128:(kk + 1) * 128].astype(BF)

        # hist8[p, q, j, u] = histp[us].T[(2q+j)*128+p, u] * SHI
        hpad = np.zeros((TH * 2 * 128, UCP), np.float32)
        hpad[:H, :UC] = histp[us].T * SHI
        hist8 = np.ascontiguousarray(
            hpad.reshape(TH, 2, 128, UCP).transpose(2, 0, 1, 3)
            .reshape(128, TH * 2 * UCP)).astype(E4)

        # cuv: cu replicated across partitions (bf16), for the sT column scale
        cuv = np.zeros((128, UCP), BF)
        cuv[:, :UC] = np.broadcast_to(cu_c[None, :], (128, UC)).astype(BF)

        # crow row 0 = 1/ci (qT bias row), row 1 = bu + gm (sT bias row)
        crow = np.zeros((2, UCP), np.float32)
        crow[0, :I] = 1.0 / ci
        crow[1, :UC] = bu[us, 0] + gm

        in_maps.append({
            "ga8": ga8, "gb8": gb8, "hist8": hist8, "y08": y08,
            "uftcat": uftcat, "iftcat": iftcat,
            "wcat": wcat, "fcwb": fcwb, "cpack": cpack, "crow": crow,
            "cuv": cuv,
        })
    return in_maps


def declare_io(nc, timing_mode=False):
    t = {}
    def inp(name, shape, dt):
        t[name] = nc.dram_tensor(name, list(shape), dt, kind="ExternalInput").ap()
    inp("ga8", (TU, 128, R * 2 * IP), FP8E4)
    inp("gb8", (TI, 128, R * 2 * UCP), FP8E4)
    inp("hist8", (128, TH * 2 * UCP), FP8E4)
    inp("y08", (128, TH * 2 * O), FP8E4)
    inp("uftcat", (128, KD * UCP), BF16)
    inp("iftcat", (128, KD * IP), BF16)
    inp("wcat", (128, KD * RA), BF16)
    inp("fcwb", (128, KD * O), BF16)
    inp("cpack", (128, 17), F32)
    inp("crow", (2, UCP), F32)
    inp("cuv", (128, UCP), BF16)
    if timing_mode:
        t["tick"] = nc.dram_tensor("tick", [1, 4], BF16, kind="ExternalOutput").ap()
    else:
        t["out"] = nc.dram_tensor("out", [128, KI * UC], BF16,
                                  kind="ExternalOutput").ap()
    return t


def emit_body(nc, tc, t, it, timing_mode=False, loop_mode=False,
              phases="ABHCDE"):
    """Emit one full compute pass. `it` suffixes tile names for repeats.

    `phases` selects blocks (HW phase-timing probes): A=user XW+fp8 evict,
    B=item partials, H=item-side XW, C=user aggregates, D=heads,
    E=final matmul+out (e = matmul only, no out DMA).
    """
    from contextlib import ExitStack
    ctx = ExitStack()
    P = 128
    phA, phB, phH = "A" in phases, "B" in phases, "H" in phases
    phC, phD = "C" in phases, "D" in phases
    phE = "E" in phases or "e" in phases
    phEdma = "E" in phases

    const = ctx.enter_context(tc.tile_pool(name=f"const{it}", bufs=1))

    def loadc(name, shape, dt, src_ap, eng):
        tl = const.tile(shape, dt, name=f"{name}{it}")
        eng.dma_start(tl[:], src_ap)
        return tl

    # sync queue: uft, wc, ga8*5, itemp/iag; scalar queue: ift, fcw, cp, y08,
    # gb8*4, hist8, out; gpsimd: crow, cuv.
    uft = loadc("uft", [P, KD * UCP], BF16, t["uftcat"][:], nc.sync)
    wc = loadc("wc", [P, KD * RA], BF16, t["wcat"][:], nc.sync)
    ift = loadc("ift", [P, KD * IP], BF16, t["iftcat"][:], nc.scalar)
    fcw = loadc("fcw", [P, KD * O], BF16, t["fcwb"][:], nc.scalar)
    cp = loadc("cp", [P, 17], F32, t["cpack"][:], nc.scalar)
    y08t = loadc("y08t", [P, TH, 2, O], FP8E4, t["y08"][:], nc.scalar)
    cr0 = const.tile([1, UCP], F32, name=f"cr0{it}")
    cr1 = const.tile([1, UCP], F32, name=f"cr1{it}")
    cuv = loadc("cuv", [P, UCP], BF16, t["cuv"][:], nc.gpsimd)

    ci2 = cp[:, 0:KI]
    bi2 = cp[:, KI:2 * KI]
    fcb = cp[0:O, 16:17]

    ga_t = []
    for tt in range(TU):
        g = const.tile([P, R, 2, IP], FP8E4, name=f"ga_t{tt}{it}")
        nc.sync.dma_start(g[:], t["ga8"][tt])
        ga_t.append(g)
    gb_t = []
    for jt in range(TI):
        g = const.tile([P, R, 2, UCP], FP8E4, name=f"gb_t{jt}{it}")
        nc.scalar.dma_start(g[:], t["gb8"][jt])
        gb_t.append(g)
    hist = const.tile([P, TH, 2, UCP], FP8E4, name=f"hist{it}")
    nc.scalar.dma_start(hist[:], t["hist8"][:])

    # alternate PSUM->SBUF evictions between DVE and ACT
    _evict_flip = [0]
    def evict8(dst_ap, src_ap, scale):
        if _evict_flip[0] % 2 == 0:
            nc.scalar.activation(dst_ap, src_ap, _ACT.Identity, scale=scale)
        else:
            nc.vector.tensor_scalar(dst_ap, src_ap, scale, None, _ALU.mult)
        _evict_flip[0] += 1

    # ---------------- phase A: user XW -> xh8 (fp8, scale SX) --------------
    xh8 = [const.tile([P, 2, RA], FP8E4, name=f"xh8{tt}{it}") for tt in range(TU)]
    hh8 = [const.tile([P, 2, RA], FP8E4, name=f"hh8{jt}{it}") for jt in range(TI)]
    pa = ExitStack()
    psxw = pa.enter_context(tc.tile_pool(name=f"psxw{it}", bufs=4, space="PSUM"))

    def emit_xw(dst, j2, src_cols, scale):
        # dst[:, j2, c0:c0+cw] = fp8(scale * sum_kk src^T W)
        for (c0, cw) in RCHUNKS:
            ps = psxw.tile([P, 512], F32, name=f"psxw{it}", tag="psxw")
            for kk in range(KD):
                nc.tensor.matmul(ps[:, 0:cw], src_cols[kk],
                                 wc[:, kk * RA + c0:kk * RA + c0 + cw],
                                 start=(kk == 0), stop=(kk == KD - 1))
            evict8(dst[:, j2, c0:c0 + cw], ps[:, 0:cw], scale)

    if phA:
        for k in range(KU):
            emit_xw(xh8[k // 2], k % 2,
                    [uft[:, kk * UCP + k * P:kk * UCP + (k + 1) * P]
                     for kk in range(KD)], SX)

    # ---------------- phase B: item partials (DoubleRow) -------------------
    dram = ctx.enter_context(tc.tile_pool(name=f"dram{it}", bufs=1, space="DRAM"))
    itemp = dram.tile([D, IP], BF16, name=f"itemp{it}")
    itemagg = dram.tile([D, IP], BF16, name=f"itemagg{it}",
                        addr_space="Local" if loop_mode else "Shared")
    mcT = [const.tile([P, IP], BF16, name=f"mcT{h}{it}") for h in range(2)]
    if phB:
        pb = ExitStack()
        psb = pb.enter_context(tc.tile_pool(name=f"psb{it}", bufs=1, space="PSUM"))
        psB = [[psb.tile([P, 512], F32, name=f"psB{h}{cix}{it}")
                for cix in range(2)] for h in range(2)]
        for tt in range(TU):
            for r in range(R):
                for h in range(2):
                    lhsT = xh8[tt][:, :, r * D + h * P:r * D + (h + 1) * P]
                    for cix, (c0, cw) in enumerate(ICHUNKS):
                        nc.tensor.matmul(psB[h][cix][:], lhsT,
                                         ga_t[tt][:, r, :, c0:c0 + cw],
                                         start=(tt == 0 and r == 0),
                                         stop=(tt == TU - 1 and r == R - 1),
                                         perf_mode=_DR)
        for h in range(2):
            for cix, (c0, cw) in enumerate(ICHUNKS):
                evict8(mcT[h][:, c0:c0 + cw], psB[h][cix][:], 1.0 / SX)
        for h in range(2):
            nc.sync.dma_start(itemp[h * P:(h + 1) * P, :], mcT[h][:])
        if not loop_mode:
            nc.gpsimd.collective_compute(
                "AllReduce", _ALU.add,
                replica_groups=[list(range(N_CORES))],
                ins=[itemp.opt()], outs=[itemagg.opt()],
            )
        pb.close()

    # ---------------- phase H: item XW -> hh8 (fp8, scale SHH) -------------
    if phH:
        for j in range(KI):
            emit_xw(hh8[j // 2], j % 2,
                    [ift[:, kk * IP + j * P:kk * IP + (j + 1) * P]
                     for kk in range(KD)], SHH)
    pa.close()

    # ---------------- q head (hidden under phase C) ------------------------
    pdq = ExitStack()
    pss = pdq.enter_context(tc.tile_pool(name=f"pss{it}", bufs=2, space="PSUM"))
    qT = const.tile([O + 1, IP], BF16, name=f"qT{it}")
    sT = const.tile([O + 1, UCP], BF16, name=f"sT{it}")
    if phD:
        nc.gpsimd.dma_start(cr0[:], t["crow"][0:1, :])
        nc.gpsimd.dma_start(cr1[:], t["crow"][1:2, :])
        iag = [const.tile([P, IP], BF16, name=f"iag{kk}{it}") for kk in range(2)]
        qact = iag
        iag_src = itemp if loop_mode else itemagg
        for kk in range(2):
            nc.sync.dma_start(iag[kk][:], iag_src[kk * P:(kk + 1) * P, :])
            if loop_mode:
                nc.sync.dma_start(itemagg[kk * P:(kk + 1) * P, :], iag[kk][:])
            nc.vector.scalar_tensor_tensor(qact[kk][:], iag[kk][:], 0.1,
                                           iag[kk][:], _ALU.mult, _ALU.max)
        for (c0, cw) in ICHUNKS:
            psQ = pss.tile([O, 512], F32, name=f"psQ{it}", tag="pss")
            for kk in range(2):
                nc.tensor.matmul(psQ[:, 0:cw], fcw[:, kk * O:(kk + 1) * O],
                                 qact[kk][:, c0:c0 + cw],
                                 start=(kk == 0), stop=(kk == 1))
            nc.scalar.activation(qT[0:O, c0:c0 + cw], psQ[:, 0:cw],
                                 _ACT.Identity, bias=fcb, scale=1.0)
        nc.vector.tensor_copy(qT[O:O + 1, :], cr0[:, 0:IP])
        nc.vector.tensor_copy(sT[O:O + 1, :], cr1[:, 0:UCP])
    else:
        nc.vector.memset(qT[:], 0.0)
        nc.vector.memset(sT[:], 0.0)

    # ---------------- phase C: user aggregates (DoubleRow, transposed) -----
    # actT computed in-place in uag (leaky via STT on the same tile)
    uag = [const.tile([P, UCP], BF16, name=f"uag{a}{it}") for a in range(2)]
    actT = uag
    if phC:
        pc = ExitStack()
        psu = pc.enter_context(tc.tile_pool(name=f"psu{it}", bufs=1, space="PSUM"))
        psU = [[psu.tile([P, 512], F32, name=f"psU{a}{ci_}{it}")
                for ci_ in range(3)] for a in range(2)]
        for jt in range(TI):
            for r in range(R):
                for a in range(2):
                    lhsT = hh8[jt][:, :, r * D + a * P:r * D + (a + 1) * P]
                    for ci_, (c0, cw) in enumerate(UCHUNKS):
                        nc.tensor.matmul(psU[a][ci_][:, 0:cw], lhsT,
                                         gb_t[jt][:, r, :, c0:c0 + cw],
                                         start=(jt == 0 and r == 0),
                                         stop=(jt == TI - 1 and r == R - 1),
                                         perf_mode=_DR)
        # leaky(user_agg^T) in place -> actT bf16 (cu applied later on sT).
        for ci_, (c0, cw) in enumerate(UCHUNKS):
            for a in range(2):
                nc.scalar.activation(uag[a][:, c0:c0 + cw],
                                     psU[a][ci_][:, 0:cw], _ACT.Identity,
                                     scale=1.0 / SHH)
                nc.vector.scalar_tensor_tensor(actT[a][:, c0:c0 + cw],
                                               uag[a][:, c0:c0 + cw], 0.1,
                                               uag[a][:, c0:c0 + cw],
                                               _ALU.mult, _ALU.max)
        pc.close()
    elif phD:
        for a in range(2):
            nc.vector.memset(actT[a][:], 0.0)
    if not phB and phD:
        for h in range(2):
            nc.vector.memset(mcT[h][:], 0.0)
            nc.sync.dma_start(itemp[h * P:(h + 1) * P, :], mcT[h][:])

    if not (phD or phE):
        pdq.close()
        if timing_mode:
            src = (actT[1] if phC else hh8[TI - 1] if phH
                   else xh8[TU - 1] if phA else mcT[1])
            tickt = const.tile([1, 4], BF16, name=f"tickt{it}")
            if src is mcT[1]:
                nc.vector.tensor_copy(tickt[:], mcT[1][0:1, 0:4])
            else:
                nc.vector.tensor_copy(tickt[:], src[0:1, 0, 0:4])
            nc.sync.dma_start(t["tick"][:], tickt[:])
        ctx.close()
        return

    # ---------------- phases D+E: heads + final, chunk-interleaved ---------
    pd = ExitStack()
    pso_pool = pd.enter_context(tc.tile_pool(name=f"pso{it}", bufs=4, space="PSUM"))
    out_t = const.tile([P, KI * UC], BF16, name=f"out_t{it}")
    tmp_pool = ctx.enter_context(tc.tile_pool(name=f"tmp{it}", bufs=1))

    if timing_mode:
        out_dst = dram.tile([128, KI * UC], BF16, name=f"outscratch{it}")
    else:
        out_dst = t["out"]
    for ci_, (c0, cw) in enumerate(UCHUNKS):
        if phD:
            psS = pss.tile([O, 512], F32, name=f"psS{it}", tag="pss")
            psh = pss.tile([O, 512], F32, name=f"psh{it}", tag="pss")
            # hist matmuls (DoubleRow fp8) into psh
            for q in range(TH):
                nc.tensor.matmul(psh[:, 0:cw], y08t[:, q, :, :],
                                 hist[:, q, :, c0:c0 + cw],
                                 start=(q == 0), stop=(q == TH - 1),
                                 perf_mode=_DR)
            for kk in range(KD):
                nc.tensor.matmul(psS[:, 0:cw], fcw[:, kk * O:(kk + 1) * O],
                                 actT[kk][:, c0:c0 + cw],
                                 start=(kk == 0), stop=(kk == KD - 1))
            # sT = cu * psS + (psh/(SHI*SY) + fcb)
            tmph = tmp_pool.tile([O, 512], BF16, name=f"tmph{it}", tag="tmph")
            nc.scalar.activation(tmph[:, 0:cw], psh[:, 0:cw], _ACT.Identity,
                                 bias=fcb, scale=1.0 / (SHI * SY))
            tmpf = tmp_pool.tile([O, 512], BF16, name=f"tmpf{it}", tag="tmpf")
            nc.vector.tensor_tensor(tmpf[:, 0:cw], psS[:, 0:cw],
                                    cuv[0:O, c0:c0 + cw], _ALU.mult)
            nc.vector.tensor_tensor(sT[0:O, c0:c0 + cw], tmpf[:, 0:cw],
                                    tmph[:, 0:cw], _ALU.add)
        vw = min(cw, max(0, UC - c0))
        if vw <= 0 or not phE:
            continue
        for mi in range(KI):
            psO = pso_pool.tile([P, 512], F32, name=f"psO{it}")
            nc.tensor.matmul(psO[:, 0:cw], qT[:, mi * P:(mi + 1) * P],
                             sT[:, c0:c0 + cw], start=True, stop=True)
            if mi % 2 == 0:
                nc.scalar.activation(out_t[:, mi * UC + c0:mi * UC + c0 + vw],
                                     psO[:, 0:vw], _ACT.Identity,
                                     bias=bi2[:, mi:mi + 1], scale=ci2[:, mi:mi + 1])
            else:
                nc.vector.tensor_scalar(out_t[:, mi * UC + c0:mi * UC + c0 + vw],
                                        psO[:, 0:vw], ci2[:, mi:mi + 1],
                                        bi2[:, mi:mi + 1], _ALU.mult, _ALU.add)
    if phEdma:
        nc.scalar.dma_start(out_dst[:], out_t[:])
    pd.close()
    pdq.close()
    if timing_mode:
        tickt = const.tile([1, 4], BF16, name=f"tickt{it}")
        nc.vector.tensor_copy(tickt[:], out_t[0:1, 0:4])
        nc.sync.dma_start(t["tick"][:], tickt[:])
    ctx.close()


_PROGRAM_CACHE = {}


def build_program(repeat=1, timing_mode=False):
    key = (repeat, timing_mode)
    if key in _PROGRAM_CACHE:
        return _PROGRAM_CACHE[key]
    nc = bacc.Bacc("TRN2", target_bir_lowering=False, debug=False,
                   num_devices=N_CORES)
    t = declare_io(nc, timing_mode)
    with tile.TileContext(nc) as tc:
        for it in range(repeat):
            emit_body(nc, tc, t, f"_i{it}" if repeat > 1 else "",
                      timing_mode=timing_mode)
    nc.compile()
    _PROGRAM_CACHE[key] = (nc, t)
    return nc, t


def build_loop_program(trips, phases="ABHCDE"):
    key = ("loop", trips, phases)
    if key in _PROGRAM_CACHE:
        return _PROGRAM_CACHE[key]
    nc = bacc.Bacc("TRN2", target_bir_lowering=False, debug=False,
                   num_devices=N_CORES)
    t = declare_io(nc, timing_mode=True)
    with tile.TileContext(nc) as tc:
        with tc.For_i(0, trips, 1):
            emit_body(nc, tc, t, "", timing_mode=True, loop_mode=True,
                      phases=phases)
    nc.compile()
    _PROGRAM_CACHE[key] = (nc, t)
    return nc, t


def kernel(**inputs):
    in_maps = host_preprocess(**inputs)
    nc, _ = build_program()
    res = bass_utils.run_bass_kernel_spmd(
        nc, in_maps, core_ids=list(range(N_CORES)), trace=False)
    outs = []
    for c in range(N_CORES):
        o = res.results[c]["out"].reshape(128, KI, UC).transpose(1, 0, 2)
        outs.append(o.reshape(KI * 128, UC)[:I])
    return np.concatenate(outs, axis=1).astype(np.float32)


# revision 14
# speedup vs baseline: 1.6773x; 1.1211x over previous
"""Trainium2 Bass kernel for GCMC-style GNN message passing (nn_Net_6425271075083).

Strategy (8 NeuronCores, users sharded 1250/core), v2:
  - Host densifies edge lists into per-rating adjacency count matrices in
    fp8e4 (counts are small ints -> exact), packed in DoubleRow pair layout
    so the PE contracts 256 rows per pass at the fp8 double rate.
  - Device phases:
      A: xh_k = (cu.ufeat)_k @ W_cat  (bf16), evicted to fp8 (scale 16)
      B: item partials mcT += DoubleRow(xh8_t, ga8_t)  -> bf16 -> AllReduce
      H: hh_j = (ci.ifeat)_j @ W_cat  (bf16), evicted to fp8 (scale 256)
      C: user aggregates psU += DoubleRow(hh8_jt, gb8_jt); leaky -> actT
      D: heads: sT = cu*(fcw^T actT) + (Y0^T histT)/(32*2048) (+fcb);
         qT = fcw^T leaky(itemagg); bias rows from crow
      E: out = D_ci (qT.T @ sT) + bi, single wide DMA out (bf16)
  - DMA split across both HWDGE queues (sync + scalar); small consts on
    gpsimd SWDGE.
"""
import numpy as np
import ml_dtypes

import concourse.bass as bass
import concourse.bacc as bacc
import concourse.mybir as mybir
import concourse.tile as tile
from concourse import bass_utils

BF = ml_dtypes.bfloat16
F32 = mybir.dt.float32
BF16 = mybir.dt.bfloat16
FP8E4 = mybir.dt.float8e4
E4 = mybir.dt.np(FP8E4)

N_CORES = 8
U, I, R, D, O, H = 10000, 1000, 5, 256, 64, 1001
UC = U // N_CORES          # 1250
UCP = 1280                 # users per core, padded
IP = 1024                  # items padded
HP = 1024                  # hist bins padded
KU = UCP // 128            # 10 user k-tiles
KI = IP // 128             # 8 item k-tiles
KH = HP // 128             # 8 hist k-tiles
KD = D // 128              # 2 feature k-tiles
TU = KU // 2               # 5 user pair-tiles (DoubleRow)
TI = KI // 2               # 4 item pair-tiles
TH = KH // 2               # 4 hist pair-tiles
RA = R * D                 # 1280
SX = 16.0                  # xh fp8 scale
SHH = 256.0                # hh fp8 scale
SHI = 32.0                 # hist fp8 scale
SY = 2048.0                # y0 fp8 scale
UCHUNKS = [(0, 512), (512, 512), (1024, 256)]   # over UCP
ICHUNKS = [(0, 512), (512, 512)]                # over IP
RCHUNKS = [(0, 512), (512, 512), (1024, 256)]   # over RA

_ALU = mybir.AluOpType
_ACT = mybir.ActivationFunctionType
_DR = mybir.MatmulPerfMode.DoubleRow


def host_preprocess(src_idx, dst_idx, implicit_matrix, sqrt_count, global_mean,
                    ufeat, ifeat, W, fc_w, fc_b, bu, bi, Y):
    """Layout/sharding only plus degree/adjacency densification; the NN math
    (feature transforms, aggregation, heads) happens on device."""
    src = np.asarray(src_idx).astype(np.int64)
    dst = np.asarray(dst_idx).astype(np.int64)
    im = np.asarray(implicit_matrix).astype(np.int64)
    sqrt_count = np.asarray(sqrt_count, np.float32)
    gm = float(np.asarray(global_mean, np.float32).reshape(1)[0])
    ufeat = np.asarray(ufeat, np.float32)
    ifeat = np.asarray(ifeat, np.float32)
    W = np.asarray(W, np.float32)
    fc_w = np.asarray(fc_w, np.float32)
    fc_b = np.asarray(fc_b, np.float32)
    bu = np.asarray(bu, np.float32)
    bi = np.asarray(bi, np.float32)
    Y = np.asarray(Y, np.float32)

    deg_u = np.bincount(src.reshape(-1), minlength=U).astype(np.float32)
    deg_i = np.bincount(dst.reshape(-1), minlength=I).astype(np.float32)
    cu = 1.0 / np.sqrt(np.maximum(deg_u, 1.0))
    ci = 1.0 / np.sqrt(np.maximum(deg_i, 1.0))

    def pack_cols(vec, ntiles):
        padded = np.zeros(128 * ntiles, np.float32)
        padded[:len(vec)] = vec
        return np.ascontiguousarray(padded.reshape(ntiles, 128).T)

    # [128, 17]: cols 0:8 ci2, 8:16 bi2, col 16 rows 0:64 = fc_b
    cpack = np.zeros((128, 17), np.float32)
    cpack[:, 0:KI] = pack_cols(ci, KI)
    cpack[:, KI:2 * KI] = pack_cols(bi[:, 0], KI)
    cpack[0:O, 16] = fc_b

    # item-side consts, replicated per core
    iftcat = np.zeros((128, KD * IP), BF)
    ifn = (ifeat * ci[:, None]).T              # [256, 1000]
    for kk in range(KD):
        iftcat[:, kk * IP:kk * IP + I] = ifn[kk * 128:(kk + 1) * 128].astype(BF)

    wcat = np.zeros((128, KD * RA), BF)
    for kk in range(KD):
        for r in range(R):
            wcat[:, kk * RA + r * D:kk * RA + (r + 1) * D] = \
                W[r, kk * 128:(kk + 1) * 128, :].astype(BF)

    fcwb = np.zeros((128, KD * O), BF)
    for kk in range(KD):
        fcwb[:, kk * O:(kk + 1) * O] = fc_w[kk * 128:(kk + 1) * 128, :].astype(BF)

    Y0 = Y.copy()
    Y0[0] = 0.0
    # y08[p, q, j, o] = Y0[(2q+j)*128+p, o] * SY   (fp8 pair layout)
    ypad = np.zeros((TH * 2 * 128, O), np.float32)
    ypad[:H] = Y0 * SY
    y08 = np.ascontiguousarray(
        ypad.reshape(TH, 2, 128, O).transpose(2, 0, 1, 3).reshape(128, TH * 2 * O)
    ).astype(E4)

    # dense adjacency counts per rating [U, I]
    G = np.zeros((R, U, I), np.float32)
    for r in range(R):
        G[r] = np.bincount(src[r] * I + dst[r], minlength=U * I).reshape(U, I)

    # implicit histogram [U, H] with 1/sqrt_count folded
    hist = np.bincount((np.arange(U)[:, None] * H + im).reshape(-1),
                       minlength=U * H).reshape(U, H).astype(np.float32)
    histp = hist / sqrt_count

    in_maps = []
    for c in range(N_CORES):
        us = slice(c * UC, (c + 1) * UC)
        cu_c = cu[us]
        Gc = G[:, us, :]                        # [R, UC, I]

        # ga8[t][p, r, j, i] = G_r[(2t+j)*128+p, i]  (raw counts, fp8 exact)
        gpad = np.zeros((R, TU * 2 * 128, IP), np.float32)
        gpad[:, :UC, :I] = Gc
        ga8 = np.ascontiguousarray(
            gpad.reshape(R, TU, 2, 128, IP).transpose(1, 3, 0, 2, 4)
            .reshape(TU, 128, R * 2 * IP)).astype(E4)

        # gb8[jt][p, r, j, u] = G_r[u, (2jt+j)*128+p]  (raw counts)
        gtp = np.zeros((R, TI * 2 * 128, UCP), np.float32)
        gtp[:, :I, :UC] = Gc.transpose(0, 2, 1)
        gb8 = np.ascontiguousarray(
            gtp.reshape(R, TI, 2, 128, UCP).transpose(1, 3, 0, 2, 4)
            .reshape(TI, 128, R * 2 * UCP)).astype(E4)

        uftcat = np.zeros((128, KD * UCP), BF)
        ufn = (ufeat[us] * cu_c[:, None]).T     # [256, UC]
        for kk in range(KD):
            uftcat[:, kk * UCP:kk * UCP + UC] = ufn[kk * 128:(kk + 1) * 128].astype(BF)

        # hist8[p, q, j, u] = histp[us].T[(2q+j)*128+p, u] * SHI
        hpad = np.zeros((TH * 2 * 128, UCP), np.float32)
        hpad[:H, :UC] = histp[us].T * SHI
        hist8 = np.ascontiguousarray(
            hpad.reshape(TH, 2, 128, UCP).transpose(2, 0, 1, 3)
            .reshape(128, TH * 2 * UCP)).astype(E4)

        # cuv: cu replicated across partitions (bf16), for the sT column scale
        cuv = np.zeros((128, UCP), BF)
        cuv[:, :UC] = np.broadcast_to(cu_c[None, :], (128, UC)).astype(BF)

        # crow row 0 = 1/ci (qT bias row), row 1 = bu + gm (sT bias row)
        crow = np.zeros((2, UCP), np.float32)
        crow[0, :I] = 1.0 / ci
        crow[1, :UC] = bu[us, 0] + gm

        in_maps.append({
            "ga8": ga8, "gb8": gb8, "hist8": hist8, "y08": y08,
            "uftcat": uftcat, "iftcat": iftcat,
            "wcat": wcat, "fcwb": fcwb, "cpack": cpack, "crow": crow,
            "cuv": cuv,
        })
    return in_maps


def declare_io(nc, timing_mode=False):
    t = {}
    def inp(name, shape, dt):
        t[name] = nc.dram_tensor(name, list(shape), dt, kind="ExternalInput").ap()
    inp("ga8", (TU, 128, R * 2 * IP), FP8E4)
    inp("gb8", (TI, 128, R * 2 * UCP), FP8E4)
    inp("hist8", (128, TH * 2 * UCP), FP8E4)
    inp("y08", (128, TH * 2 * O), FP8E4)
    inp("uftcat", (128, KD * UCP), BF16)
    inp("iftcat", (128, KD * IP), BF16)
    inp("wcat", (128, KD * RA), BF16)
    inp("fcwb", (128, KD * O), BF16)
    inp("cpack", (128, 17), F32)
    inp("crow", (2, UCP), F32)
    inp("cuv", (128, UCP), BF16)
    if timing_mode:
        t["tick"] = nc.dram_tensor("tick", [1, 4], BF16, kind="ExternalOutput").ap()
    else:
        t["out"] = nc.dram_tensor("out", [128, KI * UC], BF16,
                                  kind="ExternalOutput").ap()
    return t


def emit_body(nc, tc, t, it, timing_mode=False, loop_mode=False,
              phases="ABHCDE"):
    """Emit one full compute pass. `it` suffixes tile names for repeats.

    `phases` selects blocks (HW phase-timing probes): A=user XW+fp8 evict,
    B=item partials, H=item-side XW, C=user aggregates, D=heads,
    E=final matmul+out (e = matmul only, no out DMA).
    """
    from contextlib import ExitStack
    ctx = ExitStack()
    P = 128
    phA, phB, phH = "A" in phases, "B" in phases, "H" in phases
    phC, phD = "C" in phases, "D" in phases
    phE = "E" in phases or "e" in phases
    phEdma = "E" in phases

    const = ctx.enter_context(tc.tile_pool(name=f"const{it}", bufs=1))

    def loadc(name, shape, dt, src_ap, eng):
        tl = const.tile(shape, dt, name=f"{name}{it}")
        eng.dma_start(tl[:], src_ap)
        return tl

    # sync queue: uft, wc, ga8*5, itemp/iag; scalar queue: ift, fcw, cp, y08,
    # gb8*4, hist8, out; gpsimd: crow, cuv.
    uft = loadc("uft", [P, KD * UCP], BF16, t["uftcat"][:], nc.sync)
    wc = loadc("wc", [P, KD * RA], BF16, t["wcat"][:], nc.sync)
    ift = loadc("ift", [P, KD * IP], BF16, t["iftcat"][:], nc.scalar)
    fcw = loadc("fcw", [P, KD * O], BF16, t["fcwb"][:], nc.scalar)
    cp = loadc("cp", [P, 17], F32, t["cpack"][:], nc.scalar)
    y08t = loadc("y08t", [P, TH, 2, O], FP8E4, t["y08"][:], nc.scalar)
    cr0 = const.tile([1, UCP], F32, name=f"cr0{it}")
    cr1 = const.tile([1, UCP], F32, name=f"cr1{it}")
    cuv = loadc("cuv", [P, UCP], BF16, t["cuv"][:], nc.gpsimd)

    ci2 = cp[:, 0:KI]
    bi2 = cp[:, KI:2 * KI]
    fcb = cp[0:O, 16:17]

    ga_t = []
    for tt in range(TU):
        g = const.tile([P, R, 2, IP], FP8E4, name=f"ga_t{tt}{it}")
        nc.sync.dma_start(g[:], t["ga8"][tt])
        ga_t.append(g)
    gb_t = []
    for jt in range(TI):
        g = const.tile([P, R, 2, UCP], FP8E4, name=f"gb_t{jt}{it}")
        nc.scalar.dma_start(g[:], t["gb8"][jt])
        gb_t.append(g)
    hist = const.tile([P, TH, 2, UCP], FP8E4, name=f"hist{it}")
    nc.scalar.dma_start(hist[:], t["hist8"][:])

    # alternate PSUM->SBUF evictions between DVE and ACT
    _evict_flip = [0]
    def evict8(dst_ap, src_ap, scale):
        if _evict_flip[0] % 2 == 0:
            nc.scalar.activation(dst_ap, src_ap, _ACT.Identity, scale=scale)
        else:
            nc.vector.tensor_scalar(dst_ap, src_ap, scale, None, _ALU.mult)
        _evict_flip[0] += 1

    # ---------------- phase A: user XW -> xh8 (fp8, scale SX) --------------
    xh8 = [const.tile([P, 2, RA], FP8E4, name=f"xh8{tt}{it}") for tt in range(TU)]
    hh8 = [const.tile([P, 2, RA], FP8E4, name=f"hh8{jt}{it}") for jt in range(TI)]
    pa = ExitStack()
    psxw = pa.enter_context(tc.tile_pool(name=f"psxw{it}", bufs=4, space="PSUM"))

    def emit_xw(dst, j2, src_cols, scale):
        # dst[:, j2, c0:c0+cw] = fp8(scale * sum_kk src^T W)
        for (c0, cw) in RCHUNKS:
            ps = psxw.tile([P, 512], F32, name=f"psxw{it}", tag="psxw")
            for kk in range(KD):
                nc.tensor.matmul(ps[:, 0:cw], src_cols[kk],
                                 wc[:, kk * RA + c0:kk * RA + c0 + cw],
                                 start=(kk == 0), stop=(kk == KD - 1))
            evict8(dst[:, j2, c0:c0 + cw], ps[:, 0:cw], scale)

    if phA:
        for k in range(KU):
            emit_xw(xh8[k // 2], k % 2,
                    [uft[:, kk * UCP + k * P:kk * UCP + (k + 1) * P]
                     for kk in range(KD)], SX)

    # ---------------- phase B: item partials (DoubleRow) -------------------
    dram = ctx.enter_context(tc.tile_pool(name=f"dram{it}", bufs=1, space="DRAM"))
    itemp = dram.tile([D, IP], BF16, name=f"itemp{it}")
    itemagg = dram.tile([D, IP], BF16, name=f"itemagg{it}",
                        addr_space="Local" if loop_mode else "Shared")
    mcT = [const.tile([P, IP], BF16, name=f"mcT{h}{it}") for h in range(2)]
    if phB:
        pb = ExitStack()
        psb = pb.enter_context(tc.tile_pool(name=f"psb{it}", bufs=1, space="PSUM"))
        psB = [[psb.tile([P, 512], F32, name=f"psB{h}{cix}{it}")
                for cix in range(2)] for h in range(2)]
        for tt in range(TU):
            for r in range(R):
                for h in range(2):
                    lhsT = xh8[tt][:, :, r * D + h * P:r * D + (h + 1) * P]
                    for cix, (c0, cw) in enumerate(ICHUNKS):
                        nc.tensor.matmul(psB[h][cix][:], lhsT,
                                         ga_t[tt][:, r, :, c0:c0 + cw],
                                         start=(tt == 0 and r == 0),
                                         stop=(tt == TU - 1 and r == R - 1),
                                         perf_mode=_DR)
        for h in range(2):
            for cix, (c0, cw) in enumerate(ICHUNKS):
                evict8(mcT[h][:, c0:c0 + cw], psB[h][cix][:], 1.0 / SX)
        for h in range(2):
            nc.sync.dma_start(itemp[h * P:(h + 1) * P, :], mcT[h][:])
        if not loop_mode:
            nc.gpsimd.collective_compute(
                "AllReduce", _ALU.add,
                replica_groups=[list(range(N_CORES))],
                ins=[itemp.opt()], outs=[itemagg.opt()],
            )
        pb.close()

    # ---------------- phase H: item XW -> hh8 (fp8, scale SHH) -------------
    if phH:
        for j in range(KI):
            emit_xw(hh8[j // 2], j % 2,
                    [ift[:, kk * IP + j * P:kk * IP + (j + 1) * P]
                     for kk in range(KD)], SHH)
    pa.close()

    # ---------------- q head (hidden under phase C) ------------------------
    pdq = ExitStack()
    pss = pdq.enter_context(tc.tile_pool(name=f"pss{it}", bufs=2, space="PSUM"))
    qT = const.tile([O + 1, IP], BF16, name=f"qT{it}")
    sT = const.tile([O + 1, UCP], BF16, name=f"sT{it}")
    if phD:
        nc.gpsimd.dma_start(cr0[:], t["crow"][0:1, :])
        nc.gpsimd.dma_start(cr1[:], t["crow"][1:2, :])
        iag = [const.tile([P, IP], BF16, name=f"iag{kk}{it}") for kk in range(2)]
        qact = iag
        iag_src = itemp if loop_mode else itemagg
        for kk in range(2):
            nc.sync.dma_start(iag[kk][:], iag_src[kk * P:(kk + 1) * P, :])
            if loop_mode:
                nc.sync.dma_start(itemagg[kk * P:(kk + 1) * P, :], iag[kk][:])
            nc.vector.scalar_tensor_tensor(qact[kk][:], iag[kk][:], 0.1,
                                           iag[kk][:], _ALU.mult, _ALU.max)
        for (c0, cw) in ICHUNKS:
            psQ = pss.tile([O, 512], F32, name=f"psQ{it}", tag="pss")
            for kk in range(2):
                nc.tensor.matmul(psQ[:, 0:cw], fcw[:, kk * O:(kk + 1) * O],
                                 qact[kk][:, c0:c0 + cw],
                                 start=(kk == 0), stop=(kk == 1))
            nc.scalar.activation(qT[0:O, c0:c0 + cw], psQ[:, 0:cw],
                                 _ACT.Identity, bias=fcb, scale=1.0)
        nc.vector.tensor_copy(qT[O:O + 1, :], cr0[:, 0:IP])
        nc.vector.tensor_copy(sT[O:O + 1, :], cr1[:, 0:UCP])
    else:
        nc.vector.memset(qT[:], 0.0)
        nc.vector.memset(sT[:], 0.0)

    # ---------------- phase C: user aggregates (DoubleRow, transposed) -----
    # actT computed in-place in uag (leaky via STT on the same tile)
    uag = [const.tile([P, UCP], BF16, name=f"uag{a}{it}") for a in range(2)]
    actT = uag
    if phC:
        pc = ExitStack()
        psu = pc.enter_context(tc.tile_pool(name=f"psu{it}", bufs=1, space="PSUM"))
        psU = [[psu.tile([P, 512], F32, name=f"psU{a}{ci_}{it}")
                for ci_ in range(3)] for a in range(2)]
        for jt in range(TI):
            for r in range(R):
                for a in range(2):
                    lhsT = hh8[jt][:, :, r * D + a * P:r * D + (a + 1) * P]
                    for ci_, (c0, cw) in enumerate(UCHUNKS):
                        nc.tensor.matmul(psU[a][ci_][:, 0:cw], lhsT,
                                         gb_t[jt][:, r, :, c0:c0 + cw],
                                         start=(jt == 0 and r == 0),
                                         stop=(jt == TI - 1 and r == R - 1),
                                         perf_mode=_DR)
        # leaky(user_agg^T) in place -> actT bf16 (cu applied later on sT).
        for ci_, (c0, cw) in enumerate(UCHUNKS):
            for a in range(2):
                nc.scalar.activation(uag[a][:, c0:c0 + cw],
                                     psU[a][ci_][:, 0:cw], _ACT.Identity,
                                     scale=1.0 / SHH)
                nc.vector.scalar_tensor_tensor(actT[a][:, c0:c0 + cw],
                                               uag[a][:, c0:c0 + cw], 0.1,
                                               uag[a][:, c0:c0 + cw],
                                               _ALU.mult, _ALU.max)
        pc.close()
    elif phD:
        for a in range(2):
            nc.vector.memset(actT[a][:], 0.0)
    if not phB and phD:
        for h in range(2):
            nc.vector.memset(mcT[h][:], 0.0)
            nc.sync.dma_start(itemp[h * P:(h + 1) * P, :], mcT[h][:])

    if not (phD or phE):
        pdq.close()
        if timing_mode:
            tickt = const.tile([1, 4], BF16, name=f"tickt{it}")
            if phC:
                nc.vector.tensor_copy(tickt[:], actT[1][0:1, 0:4])
            elif phH:
                nc.vector.tensor_copy(tickt[:], hh8[TI - 1][0:1, 0, 0:4])
            elif phB:
                nc.vector.tensor_copy(tickt[:], mcT[1][0:1, 0:4])
            elif phA:
                nc.vector.tensor_copy(tickt[:], xh8[TU - 1][0:1, 0, 0:4])
            nc.sync.dma_start(t["tick"][:], tickt[:])
        ctx.close()
        return

    # ---------------- phases D+E: heads + final, chunk-interleaved ---------
    pd = ExitStack()
    pso_pool = pd.enter_context(tc.tile_pool(name=f"pso{it}", bufs=4, space="PSUM"))
    out_t = const.tile([P, KI * UC], BF16, name=f"out_t{it}")
    tmp_pool = ctx.enter_context(tc.tile_pool(name=f"tmp{it}", bufs=1))

    if timing_mode:
        out_dst = dram.tile([128, KI * UC], BF16, name=f"outscratch{it}")
    else:
        out_dst = t["out"]
    for ci_, (c0, cw) in enumerate(UCHUNKS):
        if phD:
            psS = pss.tile([O, 512], F32, name=f"psS{it}", tag="pss")
            psh = pss.tile([O, 512], F32, name=f"psh{it}", tag="pss")
            # hist matmuls (DoubleRow fp8) into psh
            for q in range(TH):
                nc.tensor.matmul(psh[:, 0:cw], y08t[:, q, :, :],
                                 hist[:, q, :, c0:c0 + cw],
                                 start=(q == 0), stop=(q == TH - 1),
                                 perf_mode=_DR)
            for kk in range(KD):
                nc.tensor.matmul(psS[:, 0:cw], fcw[:, kk * O:(kk + 1) * O],
                                 actT[kk][:, c0:c0 + cw],
                                 start=(kk == 0), stop=(kk == KD - 1))
            # sT = cu * psS + (psh/(SHI*SY) + fcb)
            tmph = tmp_pool.tile([O, 512], BF16, name=f"tmph{it}", tag="tmph")
            nc.scalar.activation(tmph[:, 0:cw], psh[:, 0:cw], _ACT.Identity,
                                 bias=fcb, scale=1.0 / (SHI * SY))
            tmpf = tmp_pool.tile([O, 512], BF16, name=f"tmpf{it}", tag="tmpf")
            nc.vector.tensor_tensor(tmpf[:, 0:cw], psS[:, 0:cw],
                                    cuv[0:O, c0:c0 + cw], _ALU.mult)
            nc.vector.tensor_tensor(sT[0:O, c0:c0 + cw], tmpf[:, 0:cw],
                                    tmph[:, 0:cw], _ALU.add)
        vw = min(cw, max(0, UC - c0))
        if vw <= 0 or not phE:
            continue
        for mi in range(KI):
            psO = pso_pool.tile([P, 512], F32, name=f"psO{it}")
            nc.tensor.matmul(psO[:, 0:cw], qT[:, mi * P:(mi + 1) * P],
                             sT[:, c0:c0 + cw], start=True, stop=True)
            if mi % 2 == 0:
                nc.scalar.activation(out_t[:, mi * UC + c0:mi * UC + c0 + vw],
                                     psO[:, 0:vw], _ACT.Identity,
                                     bias=bi2[:, mi:mi + 1], scale=ci2[:, mi:mi + 1])
            else:
                nc.vector.tensor_scalar(out_t[:, mi * UC + c0:mi * UC + c0 + vw],
                                        psO[:, 0:vw], ci2[:, mi:mi + 1],
                                        bi2[:, mi:mi + 1], _ALU.mult, _ALU.add)
    if phEdma:
        nc.scalar.dma_start(out_dst[:], out_t[:])
    pd.close()
    pdq.close()
    if timing_mode:
        tickt = const.tile([1, 4], BF16, name=f"tickt{it}")
        nc.vector.tensor_copy(tickt[:], out_t[0:1, 0:4] if phE else sT[0:1, 0:4])
        nc.sync.dma_start(t["tick"][:], tickt[:])
    ctx.close()


_PROGRAM_CACHE = {}


def build_program(repeat=1, timing_mode=False):
    key = (repeat, timing_mode)
    if key in _PROGRAM_CACHE:
        return _PROGRAM_CACHE[key]
    nc = bacc.Bacc("TRN2", target_bir_lowering=False, debug=False,
                   num_devices=N_CORES)
    t = declare_io(nc, timing_mode)
    with tile.TileContext(nc) as tc:
        for it in range(repeat):
            emit_body(nc, tc, t, f"_i{it}" if repeat > 1 else "",
                      timing_mode=timing_mode)
    nc.compile()
    _PROGRAM_CACHE[key] = (nc, t)
    return nc, t


def build_loop_program(trips, phases="ABHCDE"):
    key = ("loop", trips, phases)
    if key in _PROGRAM_CACHE:
        return _PROGRAM_CACHE[key]
    nc = bacc.Bacc("TRN2", target_bir_lowering=False, debug=False,
                   num_devices=N_CORES)
    t = declare_io(nc, timing_mode=True)
    with tile.TileContext(nc) as tc:
        with tc.For_i(0, trips, 1):
            emit_body(nc, tc, t, "", timing_mode=True, loop_mode=True,
                      phases=phases)
    nc.compile()
    _PROGRAM_CACHE[key] = (nc, t)
    return nc, t


def kernel(**inputs):
    in_maps = host_preprocess(**inputs)
    nc, _ = build_program()
    res = bass_utils.run_bass_kernel_spmd(
        nc, in_maps, core_ids=list(range(N_CORES)), trace=False)
    outs = []
    for c in range(N_CORES):
        o = res.results[c]["out"].reshape(128, KI, UC).transpose(1, 0, 2)
        outs.append(o.reshape(KI * 128, UC)[:I])
    return np.concatenate(outs, axis=1).astype(np.float32)
